# revision 48
# baseline (speedup 1.0000x reference)
"""Trainium2 Bass kernel for a 2-layer dense transformer decoder (B=2, S=2048,
D=1024, H=16, F=4096, V=32000) distributed across 8 NeuronCores.

Sharding:
  - Residual stream is sequence-sharded (512 tokens/core); LayerNorms and
    residual adds run on the local shard only.
  - Attention is tensor-parallel over heads (2 heads/core): AllGather of the
    LN1 output, per-core QKV/scores/softmax/ctx for its heads, then an
    AllToAll of raw ctx head-slices (1 MB/rank vs 8 MB ReduceScatter) and a
    local full-Wo f16 GEMM on the own-token shard.
  - FFN runs fully per-token on the local shard, entirely in f16 (weights
    replicated, no f32r conversion copies).
  - LM head is vocab-sharded (4000 cols/core) after an AllGather of the final
    LN output; host concatenates the vocab shards.

Activations are stored transposed ([feature, token]); matmuls run in f16 or
float32r (both full PE rate).

Timing methodology (test.py): the axon tunnel costs ~70-90 ms per dispatch
round-trip and ~0.4 ms per queued execution regardless of kernel content, so
"HW exec time" is measured as the marginal on-device time of one forward:
chain N executions per dispatch (output buffer threads into the next call)
for a 1-rep NEFF and an R-rep NEFF (body repeated R times on device), and
divide the wall difference by N*(R-1).  This matches what neuron-profile
would report; NTFF profiling is unavailable in this environment.

Perf notes (measured via chained-exec ablations):
  - collectives cost only ~100 us total (fake-collective A/B) — not the
    bottleneck at this scale;
  - the final AllGather is split into 4 token-quarters and the LM head
    retiled to consume one quarter per m-block, so the head GEMM starts
    after 1/4 of the gather and hides the rest;
  - attention phase C runs f16 end-to-end with double-buffered softmax
    accumulators (cs) and 4-deep score PSUM (st).
"""

import contextlib
import sys

sys.path.insert(0, "/opt/trn_rl_repo")

import numpy as np

import concourse.bass as bass  # noqa: F401
import concourse.mybir as mybir
import concourse.tile as tile
from concourse import bacc

NC_ = 8
B, S, D, H, F, V, L = 2, 2048, 1024, 16, 4096, 32000, 2
T = B * S                   # 4096 global tokens
TSH = T // NC_              # 512 tokens per core
DH = 64                     # head dim
HLOC = H // NC_             # 2 heads per core
DLOC = HLOC * DH            # 128 local head dims
VSH = V // NC_              # 4000 vocab cols per core
CT = D // 128               # 8 c-tiles of the model dim
FT = F // 128               # 32 f-tiles
KT_ALL = T // 128           # 32 global k-tiles
EPS = 1e-4
SCALE = 1.0 / np.sqrt(DH)   # 0.125
QB = 512                    # q-block == TSH == AG chunk
NBLK = 500                  # head vocab n-block (8 per core)

f32 = mybir.dt.float32
f32r = mybir.dt.float32r
f16 = mybir.dt.float16
AF = mybir.ActivationFunctionType
ALU = mybir.AluOpType


def _layer_norm(nc, tc, x_tiles, g_row, b_row, out_tiles, eps_t, ones_col,
                ones_row, nm):
    """LN over the feature (partition) axis: x_tiles [128, CT, TSH] -> out_tiles."""
    with tc.tile_pool(name=f"lnw_{nm}", bufs=1) as work, \
         tc.tile_pool(name=f"lnp_{nm}", bufs=1, space="PSUM") as ps:
        xsq = work.tile([128, CT, TSH], f32r, name=f"xsq_{nm}")
        for cp in range(CT // 2):  # paired: halves ACT op count
            nc.scalar.square(xsq[:, 2 * cp:2 * cp + 2, :],
                             x_tiles[:, 2 * cp:2 * cp + 2, :])
        sum_ps = ps.tile([1, TSH], f32, name=f"sum_{nm}")
        sq_ps = ps.tile([1, TSH], f32, name=f"sq_{nm}")
        for c in range(CT):
            nc.tensor.matmul(sum_ps[:], ones_col, x_tiles[:, c, :],
                             start=(c == 0), stop=(c == CT - 1))
            nc.tensor.matmul(sq_ps[:], ones_col, xsq[:, c, :],
                             start=(c == 0), stop=(c == CT - 1))
        mu = work.tile([1, TSH], f32, name=f"mu_{nm}")
        nc.scalar.activation(mu[:], sum_ps[:], AF.Copy, scale=1.0 / D)
        msq = work.tile([1, TSH], f32, name=f"msq_{nm}")
        nc.scalar.square(msq[:], mu[:])
        var = work.tile([1, TSH], f32, name=f"var_{nm}")
        nc.vector.scalar_tensor_tensor(var[:], sq_ps[:], 1.0 / D, msq[:],
                                       op0=ALU.mult, op1=ALU.subtract)
        sd = work.tile([1, TSH], f32, name=f"sd_{nm}")
        nc.scalar.activation(sd[:], var[:], AF.Sqrt, bias=eps_t[:])
        rr = work.tile([1, TSH], f32r, name=f"rr_{nm}")
        nc.vector.reciprocal(rr[:], sd[:])
        nbr = work.tile([1, TSH], f32r, name=f"nbr_{nm}")
        nc.vector.scalar_tensor_tensor(nbr[:], mu[:], -1.0, rr[:],
                                       op0=ALU.mult, op1=ALU.mult)
        for c in range(CT):
            db = ps.tile([128, TSH], f32, name=f"db_{nm}", tag="db", bufs=2)
            cb = ps.tile([128, TSH], f32, name=f"cb_{nm}", tag="cb", bufs=2)
            nc.tensor.matmul(db[:], g_row[:, c * 128:(c + 1) * 128], rr[:],
                             start=True, stop=True)
            nc.tensor.matmul(cb[:], g_row[:, c * 128:(c + 1) * 128], nbr[:],
                             start=True, stop=False)
            nc.tensor.matmul(cb[:], b_row[:, c * 128:(c + 1) * 128], ones_row,
                             start=False, stop=True)
            tmp = work.tile([128, TSH], f32, name=f"tmp_{nm}", tag="tmp", bufs=2)
            nc.vector.tensor_tensor(tmp[:], x_tiles[:, c, :], db[:], op=ALU.mult)
            nc.vector.tensor_tensor(out_tiles[:, c, :], tmp[:], cb[:], op=ALU.add)


def build_nc(reps=None):
    import os as _os
    _NL = int(_os.environ.get("K_L", str(L)))
    _SKIP_HEAD = _os.environ.get("K_SKIP_HEAD", "0") == "1"
    _END = _os.environ.get("K_END_AFTER", "")
    _REPS = int(reps if reps is not None else _os.environ.get("K_REPS", "1"))
    _FAKE_COLL = _os.environ.get("K_FAKE_COLL", "0")  # "", "1"/"all", "ag", "a2a", "agf"
    if _FAKE_COLL == "1":
        _FAKE_COLL = "all"
    _ASP = "Local" if _FAKE_COLL else "Shared"
    nc = bacc.Bacc("TRN2", target_bir_lowering=False, debug=False, num_devices=NC_)
    lp = nc.allow_low_precision(reason="fp32r rounding acceptable for matmul inputs")
    lp.__enter__()

    # ---- I/O ----
    x0T = nc.dram_tensor("x0T", [D, TSH], f32r, kind="ExternalInput").ap()
    mask_in = nc.dram_tensor("mask", [128, 896], f32r, kind="ExternalInput").ap()
    lyr = []
    for l in range(L):
        d = {}
        for nm, shp, dt_ in [
            ("g1row", [1, D], f32r), ("b1row", [1, D], f32r),
            ("wq", [D, DLOC], f16), ("wk", [D, DLOC], f16), ("wv", [D, DLOC], f16),
            ("wo", [D, D], f16), ("bocol", [128, CT], f32),
            ("g2row", [1, D], f32r), ("b2row", [1, D], f32r),
            ("w1", [D, F], f16), ("b1col", [128, FT], f32),
            ("w2", [F, D], f16), ("b2col", [128, CT], f32),
        ]:
            d[nm] = nc.dram_tensor(f"{nm}_l{l}", shp, dt_, kind="ExternalInput").ap()
        lyr.append(d)
    gfrow = nc.dram_tensor("gfrow", [1, D], f32r, kind="ExternalInput").ap()
    bfrow = nc.dram_tensor("bfrow", [1, D], f32r, kind="ExternalInput").ap()
    wh = nc.dram_tensor("wh", [D, VSH], f16, kind="ExternalInput").ap()
    bhrow = nc.dram_tensor("bhrow", [1, VSH], f32r, kind="ExternalInput").ap()
    logits = nc.dram_tensor("logits", [T, VSH], f16, kind="ExternalOutput").ap()

    RG = [list(range(NC_))]

    def coll(kind, in_ap, out_ap, who="ag"):
        """Collective, or (K_FAKE_COLL bench mode) local DMAs writing the same
        byte volume — isolates the network premium of the real collective."""
        if _FAKE_COLL not in ("all", who):
            nc.gpsimd.collective_compute(kind, ALU.bypass, replica_groups=RG,
                                         ins=[in_ap.opt()], outs=[out_ap.opt()])
        elif kind == "AllGather":
            for r in range(NC_):
                nc.sync.dma_start(out_ap[r], in_ap)
        else:  # AllToAll: identity shuffle, same volume
            for r in range(NC_):
                nc.sync.dma_start(out_ap[r], in_ap[r])

    with tile.TileContext(nc) as tc:
        with tc.tile_pool(name="consts", bufs=1) as consts, \
             tc.tile_pool(name="xpool", bufs=1) as xpool, \
             tc.tile_pool(name="dram", bufs=1, space="DRAM") as dram:

            maskt = consts.tile([128, 896], f32r, name="maskt")
            nc.sync.dma_start(maskt[:], mask_in[:])
            ones_col = maskt[:, 895:896]          # all-ones [128, 1]
            ones_row = maskt[0:1, 384:384 + TSH]  # all-ones [1, TSH]
            eps_t = consts.tile([1, 1], f32, name="eps_t")
            nc.vector.memset(eps_t[:], EPS)

            def emit(rep):
                sfx = f"_r{rep}" if _REPS > 1 else ""
                # residual stream versions (ping-pong slots)
                xv = [xpool.tile([128, CT, TSH], f32r, name=f"x{i}{sfx}",
                                 tag=f"x{i % 2}")
                      for i in range(2 * L + 1)]
                for c in range(CT):
                    nc.sync.dma_start(xv[0][:, c, :], x0T[c * 128:(c + 1) * 128, :])

                # DRAM bounce buffers
                ag_in = [dram.tile([D, TSH], f16, name=f"agin{l}{sfx}")
                         for l in range(L)]
                ag_out = [dram.tile([NC_, D, TSH], f16,
                                    addr_space=("Local" if _FAKE_COLL in ("all", "ag")
                                                else "Shared"),
                                    name=f"agout{l}{sfx}") for l in range(L)]
                NQ = 4  # final AG split into NQ token-quarters for overlap
                agfq_in = [dram.tile([D, TSH // NQ], f16, name=f"agfin{q}{sfx}")
                           for q in range(NQ)]
                agfq_out = [dram.tile([NC_, D, TSH // NQ], f16,
                                      addr_space=("Local" if _FAKE_COLL in ("all", "agf")
                                                  else "Shared"),
                                      name=f"agfout{q}{sfx}") for q in range(NQ)]
                a2a_in = [dram.tile([NC_, DLOC, TSH], f16, name=f"a2ain{l}{sfx}")
                          for l in range(L)]
                a2a_out = [dram.tile([NC_, DLOC, TSH], f16,
                                     name=f"a2aout{l}{sfx}") for l in range(L)]

                for l in range(_NL):
                    w = lyr[l]
                    if _END:
                        # ablation mode: reuse slots so truncated layers never
                        # read tiles a skipped phase would have written
                        x_cur, x_att, x_ffn = xv[0], xv[1], xv[2]
                    else:
                        x_cur, x_att, x_ffn = xv[2 * l], xv[2 * l + 1], xv[2 * l + 2]
                    with contextlib.ExitStack() as lctx:
                        lnw = lctx.enter_context(
                            tc.tile_pool(name=f"lnw{l}{sfx}", bufs=1))

                        g1 = lnw.tile([1, D], f32r, name=f"g1_{l}{sfx}")
                        b1 = lnw.tile([1, D], f32r, name=f"b1_{l}{sfx}")
                        nc.sync.dma_start(g1[:], w["g1row"][:])
                        nc.sync.dma_start(b1[:], w["b1row"][:])

                        # ---- Phase A: LN1 on shard + AllGather ----
                        with tc.tile_pool(name=f"h1p{l}{sfx}", bufs=1) as h1p:
                            h1 = h1p.tile([128, CT, TSH], f16, name=f"h1_{l}{sfx}")
                            _layer_norm(nc, tc, x_cur, g1, b1, h1, eps_t,
                                        ones_col, ones_row, f"l{l}a{sfx}")
                            for c in range(CT):
                                nc.sync.dma_start(
                                    ag_in[l][c * 128:(c + 1) * 128, :],
                                    h1[:, c, :])
                        coll("AllGather", ag_in[l][:], ag_out[l][:], who="ag")

                        # ---- Phase B: QKV over all tokens ----
                        wqkv = lctx.enter_context(
                            tc.tile_pool(name=f"wqkv{l}{sfx}", bufs=1))
                        actx = lctx.enter_context(contextlib.ExitStack())
                        attnp = actx.enter_context(
                            tc.tile_pool(name=f"attn{l}{sfx}", bufs=1))
                        awork = actx.enter_context(
                            tc.tile_pool(name=f"awork{l}{sfx}", bufs=1))

                        wqt = wqkv.tile([128, CT, DLOC], f16, name=f"wqt_{l}{sfx}")
                        wkt = wqkv.tile([128, CT, DLOC], f16, name=f"wkt_{l}{sfx}")
                        wvt = wqkv.tile([128, CT, DLOC], f16, name=f"wvt_{l}{sfx}")
                        wot = wqkv.tile([128, CT, D], f16, name=f"wot_{l}{sfx}")
                        for c in range(CT):
                            nc.sync.dma_start(wqt[:, c, :],
                                              w["wq"][c * 128:(c + 1) * 128, :])
                            nc.sync.dma_start(wkt[:, c, :],
                                              w["wk"][c * 128:(c + 1) * 128, :])
                            nc.sync.dma_start(wvt[:, c, :],
                                              w["wv"][c * 128:(c + 1) * 128, :])
                            nc.sync.dma_start(wot[:, c, :],
                                              w["wo"][c * 128:(c + 1) * 128, :])

                        qT = attnp.tile([DLOC, T], f16, name=f"qT_{l}{sfx}")
                        kT = attnp.tile([DLOC, T], f16, name=f"kT_{l}{sfx}")
                        vt = attnp.tile([128, KT_ALL, 132], f16, name=f"vt_{l}{sfx}")
                        ctxT = attnp.tile([DLOC, T], f16, name=f"ctxT_{l}{sfx}")
                        maskf = attnp.tile([128, 896], f16, name=f"maskf_{l}{sfx}")
                        nc.vector.tensor_copy(maskf[:], maskt[:])

                        bcd = lctx.enter_context(contextlib.ExitStack())
                        bphase = bcd.enter_context(contextlib.ExitStack())
                        hstr = bphase.enter_context(
                            tc.tile_pool(name=f"hstr{l}{sfx}", bufs=1))
                        psB = bphase.enter_context(
                            tc.tile_pool(name=f"psB{l}{sfx}", bufs=1, space="PSUM"))
                        for chunk in range(NC_):
                            hts = []
                            for c in range(CT):
                                htc = hstr.tile([128, QB], f16, name=f"ht_{l}{sfx}",
                                                tag="ht", bufs=10)
                                nc.sync.dma_start(
                                    htc[:],
                                    ag_out[l][chunk, c * 128:(c + 1) * 128, :])
                                hts.append(htc)
                            qps = psB.tile([DLOC, QB], f32, name=f"qps_{l}{sfx}",
                                           tag="qps", bufs=2)
                            kps = psB.tile([DLOC, QB], f32, name=f"kps_{l}{sfx}",
                                           tag="kps", bufs=2)
                            for c in range(CT):
                                nc.tensor.matmul(qps[:], wqt[:, c, :], hts[c][:],
                                                 start=(c == 0), stop=(c == CT - 1))
                                nc.tensor.matmul(kps[:], wkt[:, c, :], hts[c][:],
                                                 start=(c == 0), stop=(c == CT - 1))
                            nc.vector.tensor_copy(
                                qT[:, chunk * QB:(chunk + 1) * QB], qps[:])
                            nc.vector.tensor_copy(
                                kT[:, chunk * QB:(chunk + 1) * QB], kps[:])
                            for sub in range(QB // 128):
                                kt_g = chunk * 4 + sub
                                vps = psB.tile([128, DLOC], f32, name=f"vps_{l}{sfx}",
                                               tag="vps", bufs=2)
                                for c in range(CT):
                                    nc.tensor.matmul(
                                        vps[:], hts[c][:, sub * 128:(sub + 1) * 128],
                                        wvt[:, c, :], start=(c == 0),
                                        stop=(c == CT - 1))
                                for hh in range(HLOC):
                                    nc.vector.tensor_copy(
                                        vt[:, kt_g, hh * 66:hh * 66 + 64],
                                        vps[:, hh * 64:(hh + 1) * 64])
                        # softmax-denominator ones columns
                        nc.scalar.copy(
                            vt[:, :, 64:65],
                            maskt[:, 895:896].broadcast_to([128, KT_ALL, 1]))
                        nc.scalar.copy(
                            vt[:, :, 130:131],
                            maskt[:, 895:896].broadcast_to([128, KT_ALL, 1]))

                        bphase.close()  # free phase-B PSUM banks for attention
                        if _END == "B":
                            continue
                        # ---- Phase C: attention ----
                        psC = bcd.enter_context(
                            tc.tile_pool(name=f"psC{l}{sfx}", bufs=1, space="PSUM"))
                        for b in range(B):
                            for hh in range(HLOC):
                                hs = slice(hh * 64, hh * 64 + 64)
                                for qb in range(S // QB):
                                    q0g = b * S + qb * QB
                                    ktmax = 4 * (qb + 1)
                                    cs = psC.tile([65, QB], f32, name=f"cs_{l}{sfx}",
                                                  tag="cs", bufs=2)
                                    # k-tiles in pairs: two score matmuls fill
                                    # a 2-bank PSUM tile, ONE exp covers both
                                    # (1024 cols) — halves ACT op count, the
                                    # attention bottleneck on HW
                                    for kp in range(ktmax // 2):
                                        st2 = psC.tile([128, 2, QB], f32,
                                                       name=f"st_{l}{sfx}",
                                                       tag="st", bufs=2)
                                        for j in range(2):
                                            kg = b * (S // 128) + 2 * kp + j
                                            nc.tensor.matmul(
                                                st2[:, j, :],
                                                kT[hs, kg * 128:kg * 128 + 128],
                                                qT[hs, q0g:q0g + QB],
                                                start=True, stop=True)
                                        e2 = awork.tile([128, 2, QB], f16,
                                                        name=f"e_{l}{sfx}",
                                                        tag="est", bufs=3)
                                        nc.scalar.activation(e2[:], st2[:], AF.Exp,
                                                             scale=SCALE)
                                        for j in range(2):
                                            k = 2 * kp + j
                                            kg = b * (S // 128) + k
                                            if (k + 1) * 128 - 1 < qb * QB:
                                                erhs = e2[:, j, :]
                                            else:
                                                em = awork.tile([128, QB], f16,
                                                                name=f"et_{l}{sfx}",
                                                                tag="et", bufs=2)
                                                sd_ = k * 128 - qb * QB
                                                nc.vector.tensor_tensor(
                                                    em[:], e2[:, j, :],
                                                    maskf[:,
                                                          384 - sd_:384 - sd_ + QB],
                                                    op=ALU.mult)
                                                erhs = em[:]
                                            nc.tensor.matmul(
                                                cs[:],
                                                vt[:, kg, hh * 66:hh * 66 + 65],
                                                erhs, start=(k == 0),
                                                stop=(k == ktmax - 1))
                                    rcp = awork.tile([1, QB], f32r,
                                                     name=f"rcp_{l}{sfx}",
                                                     tag="rcp", bufs=2)
                                    nc.vector.reciprocal(rcp[:], cs[64:65, :])
                                    rb = psC.tile([64, QB], f32, name=f"rb_{l}{sfx}",
                                                  tag="rb", bufs=2)
                                    nc.tensor.matmul(rb[:], ones_row[:, :64], rcp[:],
                                                     start=True, stop=True)
                                    rbs = awork.tile([64, QB], f32,
                                                     name=f"rbs_{l}{sfx}",
                                                     tag="rbs", bufs=2)
                                    nc.scalar.copy(rbs[:], rb[:])
                                    nc.vector.tensor_tensor(
                                        ctxT[hs, q0g:q0g + QB], cs[:64, :], rbs[:],
                                        op=ALU.mult)

                        if _END == "C":
                            continue
                        # ---- Phase D: A2A of ctx head-slices (1 MB/rank) ----
                        for dst in range(NC_):
                            osb = awork.tile([128, QB], f16,
                                             name=f"osb_{l}{sfx}",
                                             tag="osb", bufs=3)
                            if dst % 2 == 0:
                                nc.scalar.copy(osb[:],
                                               ctxT[:, dst * QB:(dst + 1) * QB])
                            else:
                                nc.vector.tensor_copy(
                                    osb[:], ctxT[:, dst * QB:(dst + 1) * QB])
                            nc.sync.dma_start(a2a_in[l][dst, :, :], osb[:])
                        bcd.close()
                        actx.close()
                        coll("AllToAll", a2a_in[l][:], a2a_out[l][:], who="a2a")

                        if _END == "D":
                            continue
                        # ---- Phase E: local full-Wo GEMM + residual + LN2 ----
                        bocolt = lnw.tile([128, CT], f32, name=f"bocolt_{l}{sfx}")
                        nc.sync.dma_start(bocolt[:], w["bocol"][:])
                        with tc.tile_pool(name=f"ctxf{l}{sfx}", bufs=1) as ctxfp, \
                             tc.tile_pool(name=f"psE{l}{sfx}", bufs=1,
                                          space="PSUM") as psE:
                            ctxf = ctxfp.tile([128, CT, TSH], f16,
                                              name=f"ctxf_{l}{sfx}")
                            for c in range(CT):
                                nc.sync.dma_start(ctxf[:, c, :],
                                                  a2a_out[l][c, :, :])
                            for n in range(CT):
                                yps = psE.tile([128, TSH], f32,
                                               name=f"yps_{l}{sfx}",
                                               tag="yps", bufs=3)
                                for c in range(CT):
                                    nc.tensor.matmul(
                                        yps[:], wot[:, c, n * 128:(n + 1) * 128],
                                        ctxf[:, c, :], start=(c == 0),
                                        stop=(c == CT - 1))
                                nc.vector.scalar_tensor_tensor(
                                    x_att[:, n, :], yps[:], bocolt[:, n:n + 1],
                                    x_cur[:, n, :], op0=ALU.add, op1=ALU.add)
                        g2 = lnw.tile([1, D], f32r, name=f"g2_{l}{sfx}")
                        b2 = lnw.tile([1, D], f32r, name=f"b2_{l}{sfx}")
                        nc.sync.dma_start(g2[:], w["g2row"][:])
                        nc.sync.dma_start(b2[:], w["b2row"][:])
                        ffp = lctx.enter_context(
                            tc.tile_pool(name=f"ffp{l}{sfx}", bufs=1))
                        relu = ffp.tile([128, FT, TSH], f16, name=f"relu_{l}{sfx}")
                        h2ctx = lctx.enter_context(contextlib.ExitStack())
                        h2p = h2ctx.enter_context(
                            tc.tile_pool(name=f"h2p{l}{sfx}", bufs=1))
                        h2 = h2p.tile([128, CT, TSH], f16, name=f"h2_{l}{sfx}")
                        _layer_norm(nc, tc, x_att, g2, b2, h2, eps_t, ones_col,
                                    ones_row, f"l{l}b{sfx}")

                        if _END == "E":
                            continue
                        # ---- Phase F: FFN on local shard (replicated fp16 weights) ----
                        b1colt = lnw.tile([128, FT], f32, name=f"b1colt_{l}{sfx}")
                        nc.sync.dma_start(b1colt[:], w["b1col"][:])
                        b2colt = lnw.tile([128, CT], f32, name=f"b2colt_{l}{sfx}")
                        nc.sync.dma_start(b2colt[:], w["b2col"][:])
                        with tc.tile_pool(name=f"w1s{l}{sfx}", bufs=2) as w1str, \
                             tc.tile_pool(name=f"psW1{l}{sfx}", bufs=1,
                                          space="PSUM") as psW1:
                            for fb in range(8):
                                w1h = w1str.tile([128, CT, 512], f16,
                                                 name=f"w1h_{l}{sfx}", tag="w1h")
                                for c in range(CT):
                                    nc.sync.dma_start(
                                        w1h[:, c, :],
                                        w["w1"][c * 128:(c + 1) * 128,
                                                fb * 512:(fb + 1) * 512])
                                for ft_ in range(4):
                                    fg = fb * 4 + ft_
                                    fps = psW1.tile([128, TSH], f32,
                                                    name=f"fps_{l}{sfx}",
                                                    tag="fps", bufs=3)
                                    for c in range(CT):
                                        nc.tensor.matmul(
                                            fps[:],
                                            w1h[:, c, ft_ * 128:(ft_ + 1) * 128],
                                            h2[:, c, :], start=(c == 0),
                                            stop=(c == CT - 1))
                                    nc.scalar.activation(relu[:, fg, :], fps[:],
                                                         AF.Relu,
                                                         bias=b1colt[:, fg:fg + 1])
                        h2ctx.close()
                        with tc.tile_pool(name=f"w2s{l}{sfx}", bufs=3) as w2str, \
                             tc.tile_pool(name=f"psF{l}{sfx}", bufs=1,
                                          space="PSUM") as psF:
                            acc = psF.tile([128, CT, TSH], f32, name=f"ffacc_{l}{sfx}")
                            for f in range(FT):
                                w2h = w2str.tile([128, D], f16, name=f"w2h_{l}{sfx}",
                                                 tag="w2h")
                                nc.sync.dma_start(w2h[:],
                                                  w["w2"][f * 128:(f + 1) * 128, :])
                                for n in range(CT):
                                    nc.tensor.matmul(
                                        acc[:, n, :], w2h[:, n * 128:(n + 1) * 128],
                                        relu[:, f, :], start=(f == 0),
                                        stop=(f == FT - 1))
                            for n in range(CT):
                                nc.vector.scalar_tensor_tensor(
                                    x_ffn[:, n, :], acc[:, n, :], b2colt[:, n:n + 1],
                                    x_att[:, n, :], op0=ALU.add, op1=ALU.add)

                # ---- Final LN + AG + head ----
                if _SKIP_HEAD:
                    # minimal output write so the module has a logits producer
                    with tc.tile_pool(name=f"stub{sfx}", bufs=1) as stub:
                        z = stub.tile([128, NBLK], f16, name=f"zstub{sfx}")
                        nc.vector.memset(z[:], 0.0)
                        nc.sync.dma_start(logits[0:128, 0:NBLK], z[:])
                    return
                with contextlib.ExitStack() as hctx:
                    lnwf = hctx.enter_context(
                        tc.tile_pool(name=f"lnwf{sfx}", bufs=1))
                    workf = hctx.enter_context(
                        tc.tile_pool(name=f"workf{sfx}", bufs=1))
                    gf = lnwf.tile([1, D], f32r, name=f"gf{sfx}")
                    bf = lnwf.tile([1, D], f32r, name=f"bf{sfx}")
                    nc.sync.dma_start(gf[:], gfrow[:])
                    nc.sync.dma_start(bf[:], bfrow[:])
                    xf = workf.tile([128, CT, TSH], f16, name=f"xf{sfx}")
                    _layer_norm(nc, tc, xv[2 * L], gf, bf, xf, eps_t, ones_col,
                                ones_row, f"fin{sfx}")
                    QW = TSH // NQ
                    for q in range(NQ):
                        for c in range(CT):
                            nc.sync.dma_start(
                                agfq_in[q][c * 128:(c + 1) * 128, :],
                                xf[:, c, q * QW:(q + 1) * QW])
                        coll("AllGather", agfq_in[q][:], agfq_out[q][:], who="agf")

                    # bh broadcast tiles [128, 8, 500]
                    bhr = lnwf.tile([1, VSH], f32r, name=f"bhr{sfx}")
                    nc.sync.dma_start(bhr[:], bhrow[:])
                    bhrep = lnwf.tile([128, NC_, NBLK], f32, name=f"bhrep{sfx}")
                    with tc.tile_pool(name=f"psbh{sfx}", bufs=1,
                                      space="PSUM") as psbh:
                        for n in range(NC_):
                            bps = psbh.tile([128, NBLK], f32, name=f"bps{sfx}",
                                            tag="bps", bufs=2)
                            nc.tensor.matmul(bps[:], ones_row[:, :128],
                                             bhr[:, n * NBLK:(n + 1) * NBLK],
                                             start=True, stop=True)
                            nc.scalar.copy(bhrep[:, n, :], bps[:])

                    # head: wh fully SBUF-resident (loads hide under the AG);
                    # 4 super-blocks of 8 m-tiles stream xf chunks
                    xfs = hctx.enter_context(tc.tile_pool(name=f"xfs{sfx}", bufs=1))
                    whs = hctx.enter_context(tc.tile_pool(name=f"whs{sfx}", bufs=1))
                    outs = hctx.enter_context(tc.tile_pool(name=f"outs{sfx}",
                                                           bufs=4))
                    psH = hctx.enter_context(tc.tile_pool(name=f"psH{sfx}", bufs=1,
                                                          space="PSUM"))
                    whall = whs.tile([128, CT, NC_ * NBLK], f16, name=f"whall{sfx}")
                    for n in range(NC_):
                        for c in range(CT):
                            nc.sync.dma_start(
                                whall[:, c, n * NBLK:(n + 1) * NBLK],
                                wh[c * 128:(c + 1) * 128,
                                   n * NBLK:(n + 1) * NBLK])
                    for q in range(NQ):
                        # quarter q holds tokens ch*TSH + q*QW + [0, QW) for
                        # every chunk ch — 8 m-tiles of 128, available as soon
                        # as AG q lands (later AGs overlap this block's GEMMs)
                        xft = xfs.tile([128, CT, NC_, QW], f16, name=f"xft{sfx}",
                                       tag="xft", bufs=2)
                        for c in range(CT):
                            nc.sync.dma_start(
                                xft[:, c, :, :],
                                agfq_out[q][:, c * 128:(c + 1) * 128,
                                            :].transpose([1, 0, 2]))
                        for n in range(NC_):
                            for m in range(NC_):
                                hps = psH.tile([128, NBLK], f32, name=f"hps{sfx}",
                                               tag="hps", bufs=4)
                                for c in range(CT):
                                    nc.tensor.matmul(
                                        hps[:], xft[:, c, m, :],
                                        whall[:, c,
                                              n * NBLK:(n + 1) * NBLK],
                                        start=(c == 0),
                                        stop=(c == CT - 1))
                                lo = outs.tile([128, NBLK], f16, name=f"lo{sfx}",
                                               tag="lo")
                                nc.vector.tensor_tensor(lo[:], hps[:],
                                                        bhrep[:, n, :],
                                                        op=ALU.add)
                                row0 = m * TSH + q * QW
                                nc.sync.dma_start(
                                    logits[row0:row0 + QW,
                                           n * NBLK:(n + 1) * NBLK], lo[:])

            for rep in range(_REPS):
                emit(rep)

    nc.compile()
    return nc


def _host_inputs(tokens, emb, pe, ln1_g, ln1_b, Wq, Wk, Wv, Wo, bo,
                 ln2_g, ln2_b, W1, b1, W2, b2, lnf_g, lnf_b, Wh, bh):
    tokens = np.asarray(tokens)
    emb = np.asarray(emb, dtype=np.float32)
    pe = np.asarray(pe, dtype=np.float32)
    x0 = (emb[tokens] + pe[None]).reshape(T, D)  # [4096, 1024]
    mask = (np.arange(896, dtype=np.int64)[None, :] - 384
            >= np.arange(128, dtype=np.int64)[:, None]).astype(np.float32)

    def colmaj(v, nt):  # [nt*128] -> [128, nt] column tiles
        return np.ascontiguousarray(np.asarray(v, np.float32).reshape(nt, 128).T)

    Wqf = np.asarray(Wq, np.float32)
    Wkf = np.asarray(Wk, np.float32)
    Wvf = np.asarray(Wv, np.float32)
    Wof = np.asarray(Wo, np.float32)
    W1f = np.asarray(W1, np.float32)
    W2f = np.asarray(W2, np.float32)
    Whf = np.asarray(Wh, np.float32)

    in_maps = []
    for c in range(NC_):
        m = {
            "x0T": np.ascontiguousarray(x0[c * TSH:(c + 1) * TSH].T),
            "mask": mask,
            "gfrow": np.ascontiguousarray(np.asarray(lnf_g, np.float32)[None, :]),
            "bfrow": np.ascontiguousarray(np.asarray(lnf_b, np.float32)[None, :]),
            "wh": np.ascontiguousarray(Whf[:, c * VSH:(c + 1) * VSH]).astype(np.float16),
            "bhrow": np.ascontiguousarray(np.asarray(bh, np.float32)[None,
                                                                     c * VSH:(c + 1) * VSH]),
        }
        hsl = slice(c * DLOC, (c + 1) * DLOC)
        for l in range(L):
            m[f"g1row_l{l}"] = np.ascontiguousarray(np.asarray(ln1_g, np.float32)[None, l])
            m[f"b1row_l{l}"] = np.ascontiguousarray(np.asarray(ln1_b, np.float32)[None, l])
            m[f"wq_l{l}"] = np.ascontiguousarray(Wqf[l][:, hsl]).astype(np.float16)
            m[f"wk_l{l}"] = np.ascontiguousarray(Wkf[l][:, hsl]).astype(np.float16)
            m[f"wv_l{l}"] = np.ascontiguousarray(Wvf[l][:, hsl]).astype(np.float16)
            m[f"wo_l{l}"] = np.ascontiguousarray(Wof[l]).astype(np.float16)
            m[f"bocol_l{l}"] = colmaj(np.asarray(bo, np.float32)[l], CT)
            m[f"g2row_l{l}"] = np.ascontiguousarray(np.asarray(ln2_g, np.float32)[None, l])
            m[f"b2row_l{l}"] = np.ascontiguousarray(np.asarray(ln2_b, np.float32)[None, l])
            m[f"w1_l{l}"] = np.ascontiguousarray(W1f[l]).astype(np.float16)
            m[f"b1col_l{l}"] = colmaj(np.asarray(b1, np.float32)[l], FT)
            m[f"w2_l{l}"] = np.ascontiguousarray(W2f[l]).astype(np.float16)
            m[f"b2col_l{l}"] = colmaj(np.asarray(b2, np.float32)[l], CT)
        in_maps.append(m)
    return in_maps


_NC_CACHE = {}


def _get_nc():
    if "nc" not in _NC_CACHE:
        _NC_CACHE["nc"] = build_nc()
    return _NC_CACHE["nc"]


def kernel(**inputs) -> np.ndarray:
    if "runner" not in _NC_CACHE:
        _NC_CACHE["runner"] = make_runner()
    stage, run, to_numpy = _NC_CACHE["runner"][:3]
    in_maps = _host_inputs(**inputs)
    stage(in_maps)
    res = to_numpy(run())
    out = np.concatenate(
        [res[c]["logits"].reshape(B, S, VSH) for c in range(NC_)], axis=-1)
    return out.astype(np.float32)


def make_runner(reps=None):
    """Build a reusable PJRT runner: compile once, keep inputs device-resident.

    Returns (stage, run, to_numpy, run_chained). With reps=N the NEFF executes
    the full forward N times back-to-back on device (used only for timing the
    marginal per-forward device time; kernel() always uses reps=1).
    """
    import jax
    from jax.sharding import Mesh, PartitionSpec, NamedSharding
    from jax.experimental.shard_map import shard_map
    from concourse.bass2jax import (_bass_exec_p, install_neuronx_cc_hook,
                                    partition_id_tensor)
    import concourse.mybir as mybir_

    nc = build_nc(reps) if reps is not None else _get_nc()
    install_neuronx_cc_hook()

    in_names, out_names, out_avals, zero_outs = [], [], [], []
    for alloc in nc.m.functions[0].allocations:
        if not isinstance(alloc, mybir_.MemoryLocationSet):
            continue
        name = alloc.memorylocations[0].name
        if alloc.kind == "ExternalInput":
            in_names.append(name)
        elif alloc.kind == "ExternalOutput":
            out_names.append(name)
            shape = tuple(alloc.tensor_shape)
            dtype = mybir_.dt.np(alloc.dtype)
            out_avals.append(jax.core.ShapedArray(shape, dtype))
            zero_outs.append(np.zeros(shape, dtype))
    partition_name = (nc.partition_id_tensor.name
                      if nc.partition_id_tensor else None)
    if partition_name in in_names:
        in_names.remove(partition_name)
    n_params = len(in_names)
    all_names = in_names + out_names
    if partition_name is not None:
        all_names = all_names + [partition_name]

    def _body(*args):
        operands = list(args)
        if partition_name is not None:
            operands.append(partition_id_tensor())
        outs = _bass_exec_p.bind(
            *operands,
            out_avals=tuple(out_avals),
            in_names=tuple(all_names),
            out_names=tuple(out_names),
            lowering_input_output_aliases=(),
            sim_require_finite=True,
            sim_require_nnan=True,
            nc=nc,
        )
        return tuple(outs)

    devices = jax.devices()[:NC_]
    mesh = Mesh(np.asarray(devices), ("core",))
    spec = NamedSharding(mesh, PartitionSpec("core"))
    n_all = n_params + len(out_names)
    sharded = jax.jit(
        shard_map(_body, mesh=mesh,
                  in_specs=(PartitionSpec("core"),) * n_all,
                  out_specs=(PartitionSpec("core"),) * len(out_names),
                  check_rep=False),
        keep_unused=True)

    state = {}

    import jax.numpy as jnp

    def stage(in_maps):
        concat = [np.concatenate([np.asarray(in_maps[c][nm]) for c in range(NC_)],
                                 axis=0) for nm in in_names]
        dev = [jax.device_put(a, spec) for a in concat]
        zfn = jax.jit(
            lambda: tuple(jnp.zeros((NC_ * z.shape[0], *z.shape[1:]), z.dtype)
                          for z in zero_outs),
            out_shardings=(spec,) * len(zero_outs))
        zdev = list(zfn())
        for a in dev + zdev:
            a.block_until_ready()
        state["dev"] = dev + zdev
        if "compiled" not in state:
            try:
                from concourse.bass2jax import fast_dispatch_compile
                state["compiled"] = fast_dispatch_compile(
                    lambda: sharded.lower(*state["dev"]).compile())
            except Exception:
                state["compiled"] = None

    def run():
        fn = state.get("compiled") or sharded
        outs = fn(*state["dev"])
        for o in outs:
            o.block_until_ready()
        return outs

    def run_chained(n):
        """Run the executable n times back-to-back on device, chaining each
        execution's output buffer into the next call's output-seed operand.
        The data dependency makes the device run them sequentially while the
        client pays only one dispatch round-trip; returns (wall_s, outs).
        Each execution is a full forward pass writing every output element."""
        import time as _time
        fn = state.get("compiled") or sharded
        ins = state["dev"][:n_params]
        outs = tuple(state["dev"][n_params:])
        t0 = _time.time()
        for _ in range(n):
            outs = fn(*ins, *outs)
        for o in outs:
            o.block_until_ready()
        return _time.time() - t0, outs

    def to_numpy(outs):
        res = []
        for c in range(NC_):
            d = {}
            for i, nm in enumerate(out_names):
                arr = np.asarray(outs[i]).reshape(NC_, *out_avals[i].shape)
                d[nm] = arr[c]
            res.append(d)
        return res

    return stage, run, to_numpy, run_chained



# revision 50
# speedup vs baseline: 1.3996x; 1.3996x over previous
"""Trainium2 Bass kernel for a 2-layer dense transformer decoder (B=2, S=2048,
D=1024, H=16, F=4096, V=32000) distributed across 8 NeuronCores.

Sharding:
  - Residual stream is sequence-sharded (512 tokens/core); LayerNorms and
    residual adds run on the local shard only.
  - Attention is tensor-parallel over heads (2 heads/core): AllGather of the
    LN1 output, per-core QKV/scores/softmax/ctx for its heads, then an
    AllToAll of raw ctx head-slices (1 MB/rank vs 8 MB ReduceScatter) and a
    local full-Wo f16 GEMM on the own-token shard.
  - FFN runs fully per-token on the local shard, entirely in f16 (weights
    replicated, no f32r conversion copies).
  - LM head is vocab-sharded (4000 cols/core) after an AllGather of the final
    LN output; host concatenates the vocab shards.

Activations are stored transposed ([feature, token]); matmuls run in f16 or
float32r (both full PE rate).

Timing methodology (test.py): the axon tunnel costs ~70-90 ms per dispatch
round-trip and ~0.4 ms per queued execution regardless of kernel content, so
"HW exec time" is measured as the marginal on-device time of one forward:
chain N executions per dispatch (output buffer threads into the next call)
for a 1-rep NEFF and an R-rep NEFF (body repeated R times on device), and
divide the wall difference by N*(R-1).  This matches what neuron-profile
would report; NTFF profiling is unavailable in this environment.

Perf notes (measured via chained-exec ablations):
  - collectives cost only ~100 us total (fake-collective A/B) — not the
    bottleneck at this scale;
  - the final AllGather is split into 4 token-quarters and the LM head
    retiled to consume one quarter per m-block, so the head GEMM starts
    after 1/4 of the gather and hides the rest;
  - attention phase C runs f16 end-to-end with double-buffered softmax
    accumulators (cs) and 4-deep score PSUM (st).
"""

import contextlib
import sys

sys.path.insert(0, "/opt/trn_rl_repo")

import numpy as np

import concourse.bass as bass  # noqa: F401
import concourse.mybir as mybir
import concourse.tile as tile
from concourse import bacc

NC_ = 8
B, S, D, H, F, V, L = 2, 2048, 1024, 16, 4096, 32000, 2
T = B * S                   # 4096 global tokens
TSH = T // NC_              # 512 tokens per core
DH = 64                     # head dim
HLOC = H // NC_             # 2 heads per core
DLOC = HLOC * DH            # 128 local head dims
VSH = V // NC_              # 4000 vocab cols per core
CT = D // 128               # 8 c-tiles of the model dim
FT = F // 128               # 32 f-tiles
KT_ALL = T // 128           # 32 global k-tiles
EPS = 1e-4
SCALE = 1.0 / np.sqrt(DH)   # 0.125
QB = 512                    # q-block == TSH == AG chunk
NBLK = 500                  # head vocab n-block (8 per core)

f32 = mybir.dt.float32
f32r = mybir.dt.float32r
f16 = mybir.dt.float16
AF = mybir.ActivationFunctionType
ALU = mybir.AluOpType


def _layer_norm(nc, tc, x_tiles, g_row, b_row, out_tiles, eps_t, ones_col,
                ones_row, nm):
    """LN over the feature (partition) axis: x_tiles [128, CT, TSH] -> out_tiles."""
    with tc.tile_pool(name=f"lnw_{nm}", bufs=1) as work, \
         tc.tile_pool(name=f"lnp_{nm}", bufs=1, space="PSUM") as ps:
        xsq = work.tile([128, CT, TSH], f32r, name=f"xsq_{nm}")
        for c in range(CT):
            nc.scalar.square(xsq[:, c, :], x_tiles[:, c, :])
        sum_ps = ps.tile([1, TSH], f32, name=f"sum_{nm}")
        sq_ps = ps.tile([1, TSH], f32, name=f"sq_{nm}")
        for c in range(CT):
            nc.tensor.matmul(sum_ps[:], ones_col, x_tiles[:, c, :],
                             start=(c == 0), stop=(c == CT - 1))
            nc.tensor.matmul(sq_ps[:], ones_col, xsq[:, c, :],
                             start=(c == 0), stop=(c == CT - 1))
        mu = work.tile([1, TSH], f32, name=f"mu_{nm}")
        nc.scalar.activation(mu[:], sum_ps[:], AF.Copy, scale=1.0 / D)
        msq = work.tile([1, TSH], f32, name=f"msq_{nm}")
        nc.scalar.square(msq[:], mu[:])
        var = work.tile([1, TSH], f32, name=f"var_{nm}")
        nc.vector.scalar_tensor_tensor(var[:], sq_ps[:], 1.0 / D, msq[:],
                                       op0=ALU.mult, op1=ALU.subtract)
        sd = work.tile([1, TSH], f32, name=f"sd_{nm}")
        nc.scalar.activation(sd[:], var[:], AF.Sqrt, bias=eps_t[:])
        rr = work.tile([1, TSH], f32r, name=f"rr_{nm}")
        nc.vector.reciprocal(rr[:], sd[:])
        nbr = work.tile([1, TSH], f32r, name=f"nbr_{nm}")
        nc.vector.scalar_tensor_tensor(nbr[:], mu[:], -1.0, rr[:],
                                       op0=ALU.mult, op1=ALU.mult)
        for c in range(CT):
            db = ps.tile([128, TSH], f32, name=f"db_{nm}", tag="db", bufs=2)
            cb = ps.tile([128, TSH], f32, name=f"cb_{nm}", tag="cb", bufs=2)
            nc.tensor.matmul(db[:], g_row[:, c * 128:(c + 1) * 128], rr[:],
                             start=True, stop=True)
            nc.tensor.matmul(cb[:], g_row[:, c * 128:(c + 1) * 128], nbr[:],
                             start=True, stop=False)
            nc.tensor.matmul(cb[:], b_row[:, c * 128:(c + 1) * 128], ones_row,
                             start=False, stop=True)
            tmp = work.tile([128, TSH], f32, name=f"tmp_{nm}", tag="tmp", bufs=2)
            nc.vector.tensor_tensor(tmp[:], x_tiles[:, c, :], db[:], op=ALU.mult)
            nc.vector.tensor_tensor(out_tiles[:, c, :], tmp[:], cb[:], op=ALU.add)


def build_nc(reps=None):
    import os as _os
    _NL = int(_os.environ.get("K_L", str(L)))
    _SKIP_HEAD = _os.environ.get("K_SKIP_HEAD", "0") == "1"
    _END = _os.environ.get("K_END_AFTER", "")
    _REPS = int(reps if reps is not None else _os.environ.get("K_REPS", "1"))
    _FAKE_COLL = _os.environ.get("K_FAKE_COLL", "0")  # "", "1"/"all", "ag", "a2a", "agf"
    if _FAKE_COLL == "1":
        _FAKE_COLL = "all"
    _ASP = "Local" if _FAKE_COLL else "Shared"
    nc = bacc.Bacc("TRN2", target_bir_lowering=False, debug=False, num_devices=NC_)
    lp = nc.allow_low_precision(reason="fp32r rounding acceptable for matmul inputs")
    lp.__enter__()

    # ---- I/O ----
    x0T = nc.dram_tensor("x0T", [D, TSH], f32r, kind="ExternalInput").ap()
    mask_in = nc.dram_tensor("mask", [128, 896], f32r, kind="ExternalInput").ap()
    lyr = []
    for l in range(L):
        d = {}
        for nm, shp, dt_ in [
            ("g1row", [1, D], f32r), ("b1row", [1, D], f32r),
            ("wq", [D, DLOC], f16), ("wk", [D, DLOC], f16), ("wv", [D, DLOC], f16),
            ("wo", [D, D], f16), ("bocol", [128, CT], f32),
            ("g2row", [1, D], f32r), ("b2row", [1, D], f32r),
            ("w1", [D, F], f16), ("b1col", [128, FT], f32),
            ("w2", [F, D], f16), ("b2col", [128, CT], f32),
        ]:
            d[nm] = nc.dram_tensor(f"{nm}_l{l}", shp, dt_, kind="ExternalInput").ap()
        lyr.append(d)
    gfrow = nc.dram_tensor("gfrow", [1, D], f32r, kind="ExternalInput").ap()
    bfrow = nc.dram_tensor("bfrow", [1, D], f32r, kind="ExternalInput").ap()
    wh = nc.dram_tensor("wh", [D, VSH], f16, kind="ExternalInput").ap()
    bhrow = nc.dram_tensor("bhrow", [1, VSH], f32r, kind="ExternalInput").ap()
    logits = nc.dram_tensor("logits", [T, VSH], f16, kind="ExternalOutput").ap()

    RG = [list(range(NC_))]

    def coll(kind, in_ap, out_ap, who="ag"):
        """Collective, or (K_FAKE_COLL bench mode) local DMAs writing the same
        byte volume — isolates the network premium of the real collective."""
        if _FAKE_COLL not in ("all", who):
            nc.gpsimd.collective_compute(kind, ALU.bypass, replica_groups=RG,
                                         ins=[in_ap.opt()], outs=[out_ap.opt()])
        elif kind == "AllGather":
            for r in range(NC_):
                nc.sync.dma_start(out_ap[r], in_ap)
        else:  # AllToAll: identity shuffle, same volume
            for r in range(NC_):
                nc.sync.dma_start(out_ap[r], in_ap[r])

    with tile.TileContext(nc) as tc:
        with tc.tile_pool(name="consts", bufs=1) as consts, \
             tc.tile_pool(name="xpool", bufs=1) as xpool, \
             tc.tile_pool(name="dram", bufs=1, space="DRAM") as dram:

            maskt = consts.tile([128, 896], f32r, name="maskt")
            nc.sync.dma_start(maskt[:], mask_in[:])
            ones_col = maskt[:, 895:896]          # all-ones [128, 1]
            ones_row = maskt[0:1, 384:384 + TSH]  # all-ones [1, TSH]
            eps_t = consts.tile([1, 1], f32, name="eps_t")
            nc.vector.memset(eps_t[:], EPS)

            def emit(rep):
                sfx = f"_r{rep}" if _REPS > 1 else ""
                # residual stream versions (ping-pong slots)
                xv = [xpool.tile([128, CT, TSH], f32r, name=f"x{i}{sfx}",
                                 tag=f"x{i % 2}")
                      for i in range(2 * L + 1)]
                for c in range(CT):
                    nc.sync.dma_start(xv[0][:, c, :], x0T[c * 128:(c + 1) * 128, :])

                # DRAM bounce buffers
                ag_in = [dram.tile([D, TSH], f16, name=f"agin{l}{sfx}")
                         for l in range(L)]
                ag_out = [dram.tile([NC_, D, TSH], f16,
                                    addr_space=("Local" if _FAKE_COLL in ("all", "ag")
                                                else "Shared"),
                                    name=f"agout{l}{sfx}") for l in range(L)]
                NQ = 4  # final AG split into NQ token-quarters for overlap
                agfq_in = [dram.tile([D, TSH // NQ], f16, name=f"agfin{q}{sfx}")
                           for q in range(NQ)]
                agfq_out = [dram.tile([NC_, D, TSH // NQ], f16,
                                      addr_space=("Local" if _FAKE_COLL in ("all", "agf")
                                                  else "Shared"),
                                      name=f"agfout{q}{sfx}") for q in range(NQ)]
                a2a_in = [dram.tile([NC_, DLOC, TSH], f16, name=f"a2ain{l}{sfx}")
                          for l in range(L)]
                a2a_out = [dram.tile([NC_, DLOC, TSH], f16,
                                     name=f"a2aout{l}{sfx}") for l in range(L)]

                for l in range(_NL):
                    w = lyr[l]
                    if _END:
                        # ablation mode: reuse slots so truncated layers never
                        # read tiles a skipped phase would have written
                        x_cur, x_att, x_ffn = xv[0], xv[1], xv[2]
                    else:
                        x_cur, x_att, x_ffn = xv[2 * l], xv[2 * l + 1], xv[2 * l + 2]
                    with contextlib.ExitStack() as lctx:
                        lnw = lctx.enter_context(
                            tc.tile_pool(name=f"lnw{l}{sfx}", bufs=1))

                        g1 = lnw.tile([1, D], f32r, name=f"g1_{l}{sfx}")
                        b1 = lnw.tile([1, D], f32r, name=f"b1_{l}{sfx}")
                        nc.sync.dma_start(g1[:], w["g1row"][:])
                        nc.sync.dma_start(b1[:], w["b1row"][:])

                        # ---- Phase A: LN1 on shard + AllGather ----
                        with tc.tile_pool(name=f"h1p{l}{sfx}", bufs=1) as h1p:
                            h1 = h1p.tile([128, CT, TSH], f16, name=f"h1_{l}{sfx}")
                            _layer_norm(nc, tc, x_cur, g1, b1, h1, eps_t,
                                        ones_col, ones_row, f"l{l}a{sfx}")
                            for c in range(CT):
                                nc.sync.dma_start(
                                    ag_in[l][c * 128:(c + 1) * 128, :],
                                    h1[:, c, :])
                        coll("AllGather", ag_in[l][:], ag_out[l][:], who="ag")

                        # ---- Phase B: QKV over all tokens ----
                        wqkv = lctx.enter_context(
                            tc.tile_pool(name=f"wqkv{l}{sfx}", bufs=1))
                        actx = lctx.enter_context(contextlib.ExitStack())
                        attnp = actx.enter_context(
                            tc.tile_pool(name=f"attn{l}{sfx}", bufs=1))
                        awork = actx.enter_context(
                            tc.tile_pool(name=f"awork{l}{sfx}", bufs=1))

                        wqt = wqkv.tile([128, CT, DLOC], f16, name=f"wqt_{l}{sfx}")
                        wkt = wqkv.tile([128, CT, DLOC], f16, name=f"wkt_{l}{sfx}")
                        wvt = wqkv.tile([128, CT, DLOC], f16, name=f"wvt_{l}{sfx}")
                        wot = wqkv.tile([128, CT, D], f16, name=f"wot_{l}{sfx}")
                        for c in range(CT):
                            nc.sync.dma_start(wqt[:, c, :],
                                              w["wq"][c * 128:(c + 1) * 128, :])
                            nc.sync.dma_start(wkt[:, c, :],
                                              w["wk"][c * 128:(c + 1) * 128, :])
                            nc.sync.dma_start(wvt[:, c, :],
                                              w["wv"][c * 128:(c + 1) * 128, :])
                            nc.sync.dma_start(wot[:, c, :],
                                              w["wo"][c * 128:(c + 1) * 128, :])

                        qT = attnp.tile([DLOC, T], f16, name=f"qT_{l}{sfx}")
                        kT = attnp.tile([DLOC, T], f16, name=f"kT_{l}{sfx}")
                        vt = attnp.tile([128, KT_ALL, 132], f16, name=f"vt_{l}{sfx}")
                        ctxT = attnp.tile([DLOC, T], f16, name=f"ctxT_{l}{sfx}")
                        maskf = attnp.tile([128, 896], f16, name=f"maskf_{l}{sfx}")
                        nc.vector.tensor_copy(maskf[:], maskt[:])

                        bcd = lctx.enter_context(contextlib.ExitStack())
                        bphase = bcd.enter_context(contextlib.ExitStack())
                        hstr = bphase.enter_context(
                            tc.tile_pool(name=f"hstr{l}{sfx}", bufs=1))
                        psB = bphase.enter_context(
                            tc.tile_pool(name=f"psB{l}{sfx}", bufs=1, space="PSUM"))
                        for chunk in range(NC_):
                            hts = []
                            for c in range(CT):
                                htc = hstr.tile([128, QB], f16, name=f"ht_{l}{sfx}",
                                                tag="ht", bufs=10)
                                nc.sync.dma_start(
                                    htc[:],
                                    ag_out[l][chunk, c * 128:(c + 1) * 128, :])
                                hts.append(htc)
                            qps = psB.tile([DLOC, QB], f32, name=f"qps_{l}{sfx}",
                                           tag="qps", bufs=2)
                            kps = psB.tile([DLOC, QB], f32, name=f"kps_{l}{sfx}",
                                           tag="kps", bufs=2)
                            for c in range(CT):
                                nc.tensor.matmul(qps[:], wqt[:, c, :], hts[c][:],
                                                 start=(c == 0), stop=(c == CT - 1))
                                nc.tensor.matmul(kps[:], wkt[:, c, :], hts[c][:],
                                                 start=(c == 0), stop=(c == CT - 1))
                            nc.vector.tensor_copy(
                                qT[:, chunk * QB:(chunk + 1) * QB], qps[:])
                            nc.vector.tensor_copy(
                                kT[:, chunk * QB:(chunk + 1) * QB], kps[:])
                            for sub in range(QB // 128):
                                kt_g = chunk * 4 + sub
                                vps = psB.tile([128, DLOC], f32, name=f"vps_{l}{sfx}",
                                               tag="vps", bufs=2)
                                for c in range(CT):
                                    nc.tensor.matmul(
                                        vps[:], hts[c][:, sub * 128:(sub + 1) * 128],
                                        wvt[:, c, :], start=(c == 0),
                                        stop=(c == CT - 1))
                                for hh in range(HLOC):
                                    nc.vector.tensor_copy(
                                        vt[:, kt_g, hh * 66:hh * 66 + 64],
                                        vps[:, hh * 64:(hh + 1) * 64])
                        # softmax-denominator ones columns
                        nc.scalar.copy(
                            vt[:, :, 64:65],
                            maskt[:, 895:896].broadcast_to([128, KT_ALL, 1]))
                        nc.scalar.copy(
                            vt[:, :, 130:131],
                            maskt[:, 895:896].broadcast_to([128, KT_ALL, 1]))

                        bphase.close()  # free phase-B PSUM banks for attention
                        if _END == "B":
                            continue
                        # ---- Phase C: attention ----
                        psC = bcd.enter_context(
                            tc.tile_pool(name=f"psC{l}{sfx}", bufs=1, space="PSUM"))
                        for b in range(B):
                            for qb in range(S // QB):
                                q0g = b * S + qb * QB
                                ktmax = 4 * (qb + 1)
                                for hh in range(HLOC):
                                    hs = slice(hh * 64, hh * 64 + 64)
                                    cs = psC.tile([65, QB], f32, name=f"cs_{l}{sfx}",
                                                  tag="cs", bufs=2)
                                    for k in range(ktmax):
                                        kg = b * (S // 128) + k
                                        st = psC.tile([128, QB], f32,
                                                      name=f"st_{l}{sfx}",
                                                      tag="st", bufs=4)
                                        nc.tensor.matmul(
                                            st[:], kT[hs, kg * 128:kg * 128 + 128],
                                            qT[hs, q0g:q0g + QB],
                                            start=True, stop=True)
                                        e = awork.tile([128, QB], f16,
                                                       name=f"e_{l}{sfx}",
                                                       tag="est", bufs=6)
                                        if (k + 1) * 128 - 1 < qb * QB:
                                            nc.scalar.activation(e[:], st[:], AF.Exp,
                                                                 scale=SCALE)
                                        else:
                                            et = awork.tile([128, QB], f16,
                                                            name=f"et_{l}{sfx}",
                                                            tag="et", bufs=2)
                                            nc.scalar.activation(et[:], st[:],
                                                                 AF.Exp,
                                                                 scale=SCALE)
                                            sd_ = k * 128 - qb * QB
                                            nc.vector.tensor_tensor(
                                                e[:], et[:],
                                                maskf[:, 384 - sd_:384 - sd_ + QB],
                                                op=ALU.mult)
                                        nc.tensor.matmul(
                                            cs[:], vt[:, kg, hh * 66:hh * 66 + 65],
                                            e[:], start=(k == 0),
                                            stop=(k == ktmax - 1))
                                    rcp = awork.tile([1, QB], f32r,
                                                     name=f"rcp_{l}{sfx}",
                                                     tag="rcp", bufs=2)
                                    nc.vector.reciprocal(rcp[:], cs[64:65, :])
                                    rb = psC.tile([64, QB], f32, name=f"rb_{l}{sfx}",
                                                  tag="rb", bufs=2)
                                    nc.tensor.matmul(rb[:], ones_row[:, :64], rcp[:],
                                                     start=True, stop=True)
                                    rbs = awork.tile([64, QB], f32,
                                                     name=f"rbs_{l}{sfx}",
                                                     tag="rbs", bufs=2)
                                    nc.scalar.copy(rbs[:], rb[:])
                                    nc.vector.tensor_tensor(
                                        ctxT[hs, q0g:q0g + QB], cs[:64, :], rbs[:],
                                        op=ALU.mult)

                        if _END == "C":
                            continue
                        # ---- Phase D: A2A of ctx head-slices (1 MB/rank) ----
                        for dst in range(NC_):
                            osb = awork.tile([128, QB], f16,
                                             name=f"osb_{l}{sfx}",
                                             tag="osb", bufs=3)
                            if dst % 2 == 0:
                                nc.scalar.copy(osb[:],
                                               ctxT[:, dst * QB:(dst + 1) * QB])
                            else:
                                nc.vector.tensor_copy(
                                    osb[:], ctxT[:, dst * QB:(dst + 1) * QB])
                            nc.sync.dma_start(a2a_in[l][dst, :, :], osb[:])
                        bcd.close()
                        actx.close()
                        coll("AllToAll", a2a_in[l][:], a2a_out[l][:], who="a2a")

                        if _END == "D":
                            continue
                        # ---- Phase E: local full-Wo GEMM + residual + LN2 ----
                        bocolt = lnw.tile([128, CT], f32, name=f"bocolt_{l}{sfx}")
                        nc.sync.dma_start(bocolt[:], w["bocol"][:])
                        with tc.tile_pool(name=f"ctxf{l}{sfx}", bufs=1) as ctxfp, \
                             tc.tile_pool(name=f"psE{l}{sfx}", bufs=1,
                                          space="PSUM") as psE:
                            ctxf = ctxfp.tile([128, CT, TSH], f16,
                                              name=f"ctxf_{l}{sfx}")
                            for c in range(CT):
                                nc.sync.dma_start(ctxf[:, c, :],
                                                  a2a_out[l][c, :, :])
                            for n in range(CT):
                                yps = psE.tile([128, TSH], f32,
                                               name=f"yps_{l}{sfx}",
                                               tag="yps", bufs=3)
                                for c in range(CT):
                                    nc.tensor.matmul(
                                        yps[:], wot[:, c, n * 128:(n + 1) * 128],
                                        ctxf[:, c, :], start=(c == 0),
                                        stop=(c == CT - 1))
                                nc.vector.scalar_tensor_tensor(
                                    x_att[:, n, :], yps[:], bocolt[:, n:n + 1],
                                    x_cur[:, n, :], op0=ALU.add, op1=ALU.add)
                        g2 = lnw.tile([1, D], f32r, name=f"g2_{l}{sfx}")
                        b2 = lnw.tile([1, D], f32r, name=f"b2_{l}{sfx}")
                        nc.sync.dma_start(g2[:], w["g2row"][:])
                        nc.sync.dma_start(b2[:], w["b2row"][:])
                        ffp = lctx.enter_context(
                            tc.tile_pool(name=f"ffp{l}{sfx}", bufs=1))
                        relu = ffp.tile([128, FT, TSH], f16, name=f"relu_{l}{sfx}")
                        h2ctx = lctx.enter_context(contextlib.ExitStack())
                        h2p = h2ctx.enter_context(
                            tc.tile_pool(name=f"h2p{l}{sfx}", bufs=1))
                        h2 = h2p.tile([128, CT, TSH], f16, name=f"h2_{l}{sfx}")
                        _layer_norm(nc, tc, x_att, g2, b2, h2, eps_t, ones_col,
                                    ones_row, f"l{l}b{sfx}")

                        if _END == "E":
                            continue
                        # ---- Phase F: FFN on local shard (replicated fp16 weights) ----
                        b1colt = lnw.tile([128, FT], f32, name=f"b1colt_{l}{sfx}")
                        nc.sync.dma_start(b1colt[:], w["b1col"][:])
                        b2colt = lnw.tile([128, CT], f32, name=f"b2colt_{l}{sfx}")
                        nc.sync.dma_start(b2colt[:], w["b2col"][:])
                        with tc.tile_pool(name=f"w1s{l}{sfx}", bufs=2) as w1str, \
                             tc.tile_pool(name=f"psW1{l}{sfx}", bufs=1,
                                          space="PSUM") as psW1:
                            for fb in range(8):
                                w1h = w1str.tile([128, CT, 512], f16,
                                                 name=f"w1h_{l}{sfx}", tag="w1h")
                                for c in range(CT):
                                    nc.sync.dma_start(
                                        w1h[:, c, :],
                                        w["w1"][c * 128:(c + 1) * 128,
                                                fb * 512:(fb + 1) * 512])
                                for ft_ in range(4):
                                    fg = fb * 4 + ft_
                                    fps = psW1.tile([128, TSH], f32,
                                                    name=f"fps_{l}{sfx}",
                                                    tag="fps", bufs=3)
                                    for c in range(CT):
                                        nc.tensor.matmul(
                                            fps[:],
                                            w1h[:, c, ft_ * 128:(ft_ + 1) * 128],
                                            h2[:, c, :], start=(c == 0),
                                            stop=(c == CT - 1))
                                    nc.scalar.activation(relu[:, fg, :], fps[:],
                                                         AF.Relu,
                                                         bias=b1colt[:, fg:fg + 1])
                        h2ctx.close()
                        with tc.tile_pool(name=f"w2s{l}{sfx}", bufs=3) as w2str, \
                             tc.tile_pool(name=f"psF{l}{sfx}", bufs=1,
                                          space="PSUM") as psF:
                            acc = psF.tile([128, CT, TSH], f32, name=f"ffacc_{l}{sfx}")
                            for f in range(FT):
                                w2h = w2str.tile([128, D], f16, name=f"w2h_{l}{sfx}",
                                                 tag="w2h")
                                nc.sync.dma_start(w2h[:],
                                                  w["w2"][f * 128:(f + 1) * 128, :])
                                for n in range(CT):
                                    nc.tensor.matmul(
                                        acc[:, n, :], w2h[:, n * 128:(n + 1) * 128],
                                        relu[:, f, :], start=(f == 0),
                                        stop=(f == FT - 1))
                            for n in range(CT):
                                nc.vector.scalar_tensor_tensor(
                                    x_ffn[:, n, :], acc[:, n, :], b2colt[:, n:n + 1],
                                    x_att[:, n, :], op0=ALU.add, op1=ALU.add)

                # ---- Final LN + AG + head ----
                if _SKIP_HEAD:
                    # minimal output write so the module has a logits producer
                    with tc.tile_pool(name=f"stub{sfx}", bufs=1) as stub:
                        z = stub.tile([128, NBLK], f16, name=f"zstub{sfx}")
                        nc.vector.memset(z[:], 0.0)
                        nc.sync.dma_start(logits[0:128, 0:NBLK], z[:])
                    return
                with contextlib.ExitStack() as hctx:
                    lnwf = hctx.enter_context(
                        tc.tile_pool(name=f"lnwf{sfx}", bufs=1))
                    workf = hctx.enter_context(
                        tc.tile_pool(name=f"workf{sfx}", bufs=1))
                    gf = lnwf.tile([1, D], f32r, name=f"gf{sfx}")
                    bf = lnwf.tile([1, D], f32r, name=f"bf{sfx}")
                    nc.sync.dma_start(gf[:], gfrow[:])
                    nc.sync.dma_start(bf[:], bfrow[:])
                    xf = workf.tile([128, CT, TSH], f16, name=f"xf{sfx}")
                    _layer_norm(nc, tc, xv[2 * L], gf, bf, xf, eps_t, ones_col,
                                ones_row, f"fin{sfx}")
                    QW = TSH // NQ
                    for q in range(NQ):
                        for c in range(CT):
                            nc.sync.dma_start(
                                agfq_in[q][c * 128:(c + 1) * 128, :],
                                xf[:, c, q * QW:(q + 1) * QW])
                        coll("AllGather", agfq_in[q][:], agfq_out[q][:], who="agf")

                    # bh broadcast tiles [128, 8, 500]
                    bhr = lnwf.tile([1, VSH], f32r, name=f"bhr{sfx}")
                    nc.sync.dma_start(bhr[:], bhrow[:])
                    bhrep = lnwf.tile([128, NC_, NBLK], f32, name=f"bhrep{sfx}")
                    with tc.tile_pool(name=f"psbh{sfx}", bufs=1,
                                      space="PSUM") as psbh:
                        for n in range(NC_):
                            bps = psbh.tile([128, NBLK], f32, name=f"bps{sfx}",
                                            tag="bps", bufs=2)
                            nc.tensor.matmul(bps[:], ones_row[:, :128],
                                             bhr[:, n * NBLK:(n + 1) * NBLK],
                                             start=True, stop=True)
                            nc.scalar.copy(bhrep[:, n, :], bps[:])

                    # head: wh fully SBUF-resident (loads hide under the AG);
                    # 4 super-blocks of 8 m-tiles stream xf chunks
                    xfs = hctx.enter_context(tc.tile_pool(name=f"xfs{sfx}", bufs=1))
                    whs = hctx.enter_context(tc.tile_pool(name=f"whs{sfx}", bufs=1))
                    outs = hctx.enter_context(tc.tile_pool(name=f"outs{sfx}",
                                                           bufs=4))
                    psH = hctx.enter_context(tc.tile_pool(name=f"psH{sfx}", bufs=1,
                                                          space="PSUM"))
                    whall = whs.tile([128, CT, NC_ * NBLK], f16, name=f"whall{sfx}")
                    for n in range(NC_):
                        for c in range(CT):
                            nc.sync.dma_start(
                                whall[:, c, n * NBLK:(n + 1) * NBLK],
                                wh[c * 128:(c + 1) * 128,
                                   n * NBLK:(n + 1) * NBLK])
                    for q in range(NQ):
                        # quarter q holds tokens ch*TSH + q*QW + [0, QW) for
                        # every chunk ch — 8 m-tiles of 128, available as soon
                        # as AG q lands (later AGs overlap this block's GEMMs)
                        xft = xfs.tile([128, CT, NC_, QW], f16, name=f"xft{sfx}",
                                       tag="xft", bufs=2)
                        for c in range(CT):
                            nc.sync.dma_start(
                                xft[:, c, :, :],
                                agfq_out[q][:, c * 128:(c + 1) * 128,
                                            :].transpose([1, 0, 2]))
                        for n in range(NC_):
                            for m in range(NC_):
                                hps = psH.tile([128, NBLK], f32, name=f"hps{sfx}",
                                               tag="hps", bufs=4)
                                for c in range(CT):
                                    nc.tensor.matmul(
                                        hps[:], xft[:, c, m, :],
                                        whall[:, c,
                                              n * NBLK:(n + 1) * NBLK],
                                        start=(c == 0),
                                        stop=(c == CT - 1))
                                lo = outs.tile([128, NBLK], f16, name=f"lo{sfx}",
                                               tag="lo")
                                nc.vector.tensor_tensor(lo[:], hps[:],
                                                        bhrep[:, n, :],
                                                        op=ALU.add)
                                row0 = m * TSH + q * QW
                                nc.sync.dma_start(
                                    logits[row0:row0 + QW,
                                           n * NBLK:(n + 1) * NBLK], lo[:])

            for rep in range(_REPS):
                emit(rep)

    nc.compile()
    return nc


def _host_inputs(tokens, emb, pe, ln1_g, ln1_b, Wq, Wk, Wv, Wo, bo,
                 ln2_g, ln2_b, W1, b1, W2, b2, lnf_g, lnf_b, Wh, bh):
    tokens = np.asarray(tokens)
    emb = np.asarray(emb, dtype=np.float32)
    pe = np.asarray(pe, dtype=np.float32)
    x0 = (emb[tokens] + pe[None]).reshape(T, D)  # [4096, 1024]
    mask = (np.arange(896, dtype=np.int64)[None, :] - 384
            >= np.arange(128, dtype=np.int64)[:, None]).astype(np.float32)

    def colmaj(v, nt):  # [nt*128] -> [128, nt] column tiles
        return np.ascontiguousarray(np.asarray(v, np.float32).reshape(nt, 128).T)

    Wqf = np.asarray(Wq, np.float32)
    Wkf = np.asarray(Wk, np.float32)
    Wvf = np.asarray(Wv, np.float32)
    Wof = np.asarray(Wo, np.float32)
    W1f = np.asarray(W1, np.float32)
    W2f = np.asarray(W2, np.float32)
    Whf = np.asarray(Wh, np.float32)

    in_maps = []
    for c in range(NC_):
        m = {
            "x0T": np.ascontiguousarray(x0[c * TSH:(c + 1) * TSH].T),
            "mask": mask,
            "gfrow": np.ascontiguousarray(np.asarray(lnf_g, np.float32)[None, :]),
            "bfrow": np.ascontiguousarray(np.asarray(lnf_b, np.float32)[None, :]),
            "wh": np.ascontiguousarray(Whf[:, c * VSH:(c + 1) * VSH]).astype(np.float16),
            "bhrow": np.ascontiguousarray(np.asarray(bh, np.float32)[None,
                                                                     c * VSH:(c + 1) * VSH]),
        }
        hsl = slice(c * DLOC, (c + 1) * DLOC)
        for l in range(L):
            m[f"g1row_l{l}"] = np.ascontiguousarray(np.asarray(ln1_g, np.float32)[None, l])
            m[f"b1row_l{l}"] = np.ascontiguousarray(np.asarray(ln1_b, np.float32)[None, l])
            m[f"wq_l{l}"] = np.ascontiguousarray(Wqf[l][:, hsl]).astype(np.float16)
            m[f"wk_l{l}"] = np.ascontiguousarray(Wkf[l][:, hsl]).astype(np.float16)
            m[f"wv_l{l}"] = np.ascontiguousarray(Wvf[l][:, hsl]).astype(np.float16)
            m[f"wo_l{l}"] = np.ascontiguousarray(Wof[l]).astype(np.float16)
            m[f"bocol_l{l}"] = colmaj(np.asarray(bo, np.float32)[l], CT)
            m[f"g2row_l{l}"] = np.ascontiguousarray(np.asarray(ln2_g, np.float32)[None, l])
            m[f"b2row_l{l}"] = np.ascontiguousarray(np.asarray(ln2_b, np.float32)[None, l])
            m[f"w1_l{l}"] = np.ascontiguousarray(W1f[l]).astype(np.float16)
            m[f"b1col_l{l}"] = colmaj(np.asarray(b1, np.float32)[l], FT)
            m[f"w2_l{l}"] = np.ascontiguousarray(W2f[l]).astype(np.float16)
            m[f"b2col_l{l}"] = colmaj(np.asarray(b2, np.float32)[l], CT)
        in_maps.append(m)
    return in_maps


_NC_CACHE = {}


def _get_nc():
    if "nc" not in _NC_CACHE:
        _NC_CACHE["nc"] = build_nc()
    return _NC_CACHE["nc"]


def kernel(**inputs) -> np.ndarray:
    if "runner" not in _NC_CACHE:
        _NC_CACHE["runner"] = make_runner()
    stage, run, to_numpy = _NC_CACHE["runner"][:3]
    in_maps = _host_inputs(**inputs)
    stage(in_maps)
    res = to_numpy(run())
    out = np.concatenate(
        [res[c]["logits"].reshape(B, S, VSH) for c in range(NC_)], axis=-1)
    return out.astype(np.float32)


def make_runner(reps=None):
    """Build a reusable PJRT runner: compile once, keep inputs device-resident.

    Returns (stage, run, to_numpy, run_chained). With reps=N the NEFF executes
    the full forward N times back-to-back on device (used only for timing the
    marginal per-forward device time; kernel() always uses reps=1).
    """
    import jax
    from jax.sharding import Mesh, PartitionSpec, NamedSharding
    from jax.experimental.shard_map import shard_map
    from concourse.bass2jax import (_bass_exec_p, install_neuronx_cc_hook,
                                    partition_id_tensor)
    import concourse.mybir as mybir_

    nc = build_nc(reps) if reps is not None else _get_nc()
    install_neuronx_cc_hook()

    in_names, out_names, out_avals, zero_outs = [], [], [], []
    for alloc in nc.m.functions[0].allocations:
        if not isinstance(alloc, mybir_.MemoryLocationSet):
            continue
        name = alloc.memorylocations[0].name
        if alloc.kind == "ExternalInput":
            in_names.append(name)
        elif alloc.kind == "ExternalOutput":
            out_names.append(name)
            shape = tuple(alloc.tensor_shape)
            dtype = mybir_.dt.np(alloc.dtype)
            out_avals.append(jax.core.ShapedArray(shape, dtype))
            zero_outs.append(np.zeros(shape, dtype))
    partition_name = (nc.partition_id_tensor.name
                      if nc.partition_id_tensor else None)
    if partition_name in in_names:
        in_names.remove(partition_name)
    n_params = len(in_names)
    all_names = in_names + out_names
    if partition_name is not None:
        all_names = all_names + [partition_name]

    def _body(*args):
        operands = list(args)
        if partition_name is not None:
            operands.append(partition_id_tensor())
        outs = _bass_exec_p.bind(
            *operands,
            out_avals=tuple(out_avals),
            in_names=tuple(all_names),
            out_names=tuple(out_names),
            lowering_input_output_aliases=(),
            sim_require_finite=True,
            sim_require_nnan=True,
            nc=nc,
        )
        return tuple(outs)

    devices = jax.devices()[:NC_]
    mesh = Mesh(np.asarray(devices), ("core",))
    spec = NamedSharding(mesh, PartitionSpec("core"))
    n_all = n_params + len(out_names)
    sharded = jax.jit(
        shard_map(_body, mesh=mesh,
                  in_specs=(PartitionSpec("core"),) * n_all,
                  out_specs=(PartitionSpec("core"),) * len(out_names),
                  check_rep=False),
        keep_unused=True)

    state = {}

    import jax.numpy as jnp

    def stage(in_maps):
        concat = [np.concatenate([np.asarray(in_maps[c][nm]) for c in range(NC_)],
                                 axis=0) for nm in in_names]
        dev = [jax.device_put(a, spec) for a in concat]
        zfn = jax.jit(
            lambda: tuple(jnp.zeros((NC_ * z.shape[0], *z.shape[1:]), z.dtype)
                          for z in zero_outs),
            out_shardings=(spec,) * len(zero_outs))
        zdev = list(zfn())
        for a in dev + zdev:
            a.block_until_ready()
        state["dev"] = dev + zdev
        if "compiled" not in state:
            try:
                from concourse.bass2jax import fast_dispatch_compile
                state["compiled"] = fast_dispatch_compile(
                    lambda: sharded.lower(*state["dev"]).compile())
            except Exception:
                state["compiled"] = None

    def run():
        fn = state.get("compiled") or sharded
        outs = fn(*state["dev"])
        for o in outs:
            o.block_until_ready()
        return outs

    def run_chained(n):
        """Run the executable n times back-to-back on device, chaining each
        execution's output buffer into the next call's output-seed operand.
        The data dependency makes the device run them sequentially while the
        client pays only one dispatch round-trip; returns (wall_s, outs).
        Each execution is a full forward pass writing every output element."""
        import time as _time
        fn = state.get("compiled") or sharded
        ins = state["dev"][:n_params]
        outs = tuple(state["dev"][n_params:])
        t0 = _time.time()
        for _ in range(n):
            outs = fn(*ins, *outs)
        for o in outs:
            o.block_until_ready()
        return _time.time() - t0, outs

    def to_numpy(outs):
        res = []
        for c in range(NC_):
            d = {}
            for i, nm in enumerate(out_names):
                arr = np.asarray(outs[i]).reshape(NC_, *out_avals[i].shape)
                d[nm] = arr[c]
            res.append(d)
        return res

    return stage, run, to_numpy, run_chained

